# revision 1
# baseline (speedup 1.0000x reference)
"""Trainium2 Bass kernel for an 8-head cross-attention block.

Math (per reference):
    Q = video @ Wq[h]           [4096, 64]  per head
    K = text  @ Wk[h]           [1024, 64]
    V = text  @ Wv[h]           [1024, 64]
    att = softmax(Q @ K^T)      [4096, 1024]   (no scaling)
    y_h = att @ V               [4096, 64]
    out = concat_h(y_h) @ Wout + pos_enc(4096, 512)

Sharding: head-parallel over 8 NeuronCores. Core h owns head h and the
matching 64 rows of Wout (row-parallel), producing a full [4096, 512]
partial output; the all-reduce over cores and the positional-encoding add
happen on host during the gather.

On-device layout: activations are kept "transposed" ([feature, token]) so
every TensorE contraction runs over the partition axis with no on-device
transposes. Softmax runs as exp(E - 12) (logits are O(12); the shift keeps
fp16 in range and cancels in the ratio) and the denominator comes for free
as a 65th output row of the att@V matmul via a ones-column appended to V.

Everything runs in fp16 operands (10-bit mantissa, full PE rate, half the
HBM traffic) with fp32 PSUM accumulation and an fp32 softmax denominator.
The two K=64 contraction stages (E^T and the output projection) keep their
operands duplicated on both PE partition halves so two matmuls occupy the
128-row array concurrently (row tiling via base_partition).
"""

import numpy as np

from concourse import bacc
import concourse.mybir as mybir
from concourse.tile import TileContext
from concourse.bass_utils import run_bass_kernel_spmd

N, M, D, H, DH = 4096, 1024, 512, 8, 64
P = 128
NC = 512          # n-chunk width for the attention pipeline
NJ = N // NC      # 8 n-chunks
DC = D // P       # 4 contraction chunks of 128
MT = M // P       # 8 key tiles of 128
F32 = mybir.dt.float32
FP16 = mybir.dt.float16
EXP = mybir.ActivationFunctionType.Exp
EXP_SHIFT = -12.0  # exp(E + shift): keeps exp in fp16 range; cancels in softmax
NCORES = 8

_CACHE: dict = {}
TRACE = False          # test harness can flip this before calling kernel()
LAST_RESULT = None     # BassKernelResults of the last run (for profiling)
DEBUG = False          # add intermediate dumps (dev only)


def _body(tc, nc, vT, tT, wq, wk, wv, wo, out, dscr):
    with tc.tile_pool(name="const", bufs=1) as cp:
        vt_sb = cp.tile([P, DC * N], FP16, tag="vt")
        tt_sb = cp.tile([P, DC * M], FP16, tag="tt")
        wq_sb = cp.tile([P, DC * DH], FP16, tag="wq")
        wk_sb = cp.tile([P, DC * DH], FP16, tag="wk")
        wv_sb = cp.tile([P, DC * DH], FP16, tag="wv")
        wo_sb = cp.tile([P, D], FP16, tag="wo")      # wo duplicated on both halves
        qt_sb = cp.tile([P, N], FP16, tag="qt")      # Q^T duplicated on both halves
        kt_sb = cp.tile([P, M], FP16, tag="kt")      # K^T duplicated on both halves
        v_sb = cp.tile([P, MT * (DH + 1)], FP16, tag="vsb")
        y_sb = cp.tile([P, N], FP16, tag="ysb")      # Y^T duplicated on both halves
        den_sb = cp.tile([1, N], F32, tag="den")
        rsrc = cp.tile([P, N // P], F32, tag="rsrc")
        rc_sb = cp.tile([P, N // P], F32, tag="rc")

        for c in range(DC):
            nc.sync.dma_start(out=tt_sb[:, c * M:(c + 1) * M], in_=tT[c * P:(c + 1) * P, :])
            nc.sync.dma_start(out=wq_sb[:, c * DH:(c + 1) * DH], in_=wq[c * P:(c + 1) * P, :])
            nc.sync.dma_start(out=wk_sb[:, c * DH:(c + 1) * DH], in_=wk[c * P:(c + 1) * P, :])
            nc.sync.dma_start(out=wv_sb[:, c * DH:(c + 1) * DH], in_=wv[c * P:(c + 1) * P, :])
        nc.sync.dma_start(out=wo_sb[0:DH, :], in_=wo[:, :])
        nc.sync.dma_start(out=wo_sb[DH:P, :], in_=wo[:, :])
        for c in range(DC):
            nc.sync.dma_start(out=vt_sb[:, c * N:(c + 1) * N], in_=vT[c * P:(c + 1) * P, :])

        v3 = v_sb.rearrange("p (m e) -> p m e", e=DH + 1)  # [128, 8, 65]
        nc.vector.memset(v3[:, :, DH], 1.0)
        bias_sb = cp.tile([P, 1], F32, tag="bias")
        nc.vector.memset(bias_sb[:, :], EXP_SHIFT)

        # ---- projections: K^T [64,1024], V' [128, 8x65], Q^T [64,4096] ----
        with tc.tile_pool(name="ps_proj", bufs=2, space="PSUM") as pj:
            for half in range(M // 512):
                ps = pj.tile([DH, 512], F32, tag="ps")
                for c in range(DC):
                    nc.tensor.matmul(
                        ps[:, :],
                        wk_sb[:, c * DH:(c + 1) * DH],
                        tt_sb[:, c * M + half * 512: c * M + (half + 1) * 512],
                        start=(c == 0), stop=(c == DC - 1))
                sl = slice(half * 512, (half + 1) * 512)
                nc.vector.tensor_copy(out=kt_sb[0:DH, sl], in_=ps[:, :])
                nc.vector.tensor_copy(out=kt_sb[DH:P, sl], in_=ps[:, :])
            for mt in range(MT):
                ps = pj.tile([P, DH], F32, tag="psv")
                for c in range(DC):
                    nc.tensor.matmul(
                        ps[:, :],
                        tt_sb[:, c * M + mt * P: c * M + (mt + 1) * P],
                        wv_sb[:, c * DH:(c + 1) * DH],
                        start=(c == 0), stop=(c == DC - 1))
                nc.vector.tensor_copy(out=v3[:, mt, 0:DH], in_=ps[:, :])
            for j in range(NJ):
                ps = pj.tile([DH, 512], F32, tag="ps")
                for c in range(DC):
                    nc.tensor.matmul(
                        ps[:, :],
                        wq_sb[:, c * DH:(c + 1) * DH],
                        vt_sb[:, c * N + j * NC: c * N + (j + 1) * NC],
                        start=(c == 0), stop=(c == DC - 1))
                sl = slice(j * NC, (j + 1) * NC)
                nc.vector.tensor_copy(out=qt_sb[0:DH, sl], in_=ps[:, :])
                nc.vector.tensor_copy(out=qt_sb[DH:P, sl], in_=ps[:, :])

        # ---- attention: E^T = K^T.T @ Q^T -> exp -> Y'^T = V'.T @ P^T ----
        # E matmuls are K=64: pack two per PE pass on partition halves.
        with tc.tile_pool(name="ps_e", bufs=3, space="PSUM") as pe_pool, \
             tc.tile_pool(name="ps_y", bufs=2, space="PSUM") as py_pool, \
             tc.tile_pool(name="p_sb", bufs=8) as p_pool:

            def emit_y(j, p_tiles):
                ps = py_pool.tile([DH + 1, NC], F32, tag="y")
                for mt in range(MT):
                    nc.tensor.matmul(
                        ps[:, :],
                        v3[:, mt, :],
                        p_tiles[mt // 2][:, (mt % 2) * 512:(mt % 2 + 1) * 512],
                        start=(mt == 0), stop=(mt == MT - 1))
                sl = slice(j * NC, (j + 1) * NC)
                nc.vector.tensor_copy(out=y_sb[0:DH, sl], in_=ps[0:DH, :])
                nc.vector.tensor_copy(out=y_sb[DH:P, sl], in_=ps[0:DH, :])
                # fp32 denominator row kept at full precision
                nc.vector.tensor_copy(out=den_sb[:, sl], in_=ps[DH:DH + 1, :])

            prev = None
            for j in range(NJ):
                jsl = slice(j * NC, (j + 1) * NC)
                p_tiles = []
                for pair in range(MT // 2):
                    mt = pair * 2
                    e_ps = pe_pool.tile([P, 1024], F32, tag="e")
                    nc.tensor.matmul(
                        e_ps[:, 0:512],
                        kt_sb[0:DH, mt * P:(mt + 1) * P],
                        qt_sb[0:DH, jsl],
                        start=True, stop=True)
                    nc.tensor.matmul(
                        e_ps[:, 512:1024],
                        kt_sb[DH:P, (mt + 1) * P:(mt + 2) * P],
                        qt_sb[DH:P, jsl],
                        start=True, stop=True)
                    pt = p_pool.tile([P, 1024], FP16, tag="p")
                    nc.scalar.activation(pt[:, :], e_ps[:, :], EXP, bias=bias_sb[:, :])
                    p_tiles.append(pt)
                if prev is not None:
                    emit_y(j - 1, prev)
                prev = p_tiles
            emit_y(NJ - 1, prev)

        # ---- denominator: [1, 4096] -> DRAM -> [128, 32] scatter -> recip ----
        # (a direct SBUF->SBUF partition-scatter DMA returns garbage on HW)
        nc.sync.dma_start(out=dscr[:], in_=den_sb[:, :])
        nc.sync.dma_start(out=rsrc[:, :], in_=dscr.rearrange("(t p) -> p t", p=P))
        nc.vector.reciprocal(rc_sb[:, :], rsrc[:, :])
        if DEBUG:
            dbg_den = nc.dram_tensor("dbg_den", [1, N], F32, kind="ExternalOutput")
            dbg_rsrc = nc.dram_tensor("dbg_rsrc", [P, N // P], F32, kind="ExternalOutput")
            dbg_rc = nc.dram_tensor("dbg_rc", [P, N // P], F32, kind="ExternalOutput")
            dbg_y = nc.dram_tensor("dbg_y", [P, N], FP16, kind="ExternalOutput")
            nc.sync.dma_start(out=dbg_den[:, :], in_=den_sb[:, :])
            nc.sync.dma_start(out=dbg_rsrc[:, :], in_=rsrc[:, :])
            nc.sync.dma_start(out=dbg_rc[:, :], in_=rc_sb[:, :])
            nc.sync.dma_start(out=dbg_y[:, :], in_=y_sb[:, :])

        # ---- output projection (K=64, packed two per PE pass) + scaling ----
        out_r = out.rearrange("(g p) d -> p g d", p=P)  # [128, 32, 512]
        with tc.tile_pool(name="ps_o", bufs=4, space="PSUM") as po_pool, \
             tc.tile_pool(name="o_sb", bufs=2) as o_pool:
            for g in range(N // P // 4):
                ot = o_pool.tile([P, 4 * D], FP16, tag="o")
                for k in range(0, 4, 2):
                    nt = g * 4 + k
                    ps_a = po_pool.tile([P, D], F32, tag="po")
                    ps_b = po_pool.tile([P, D], F32, tag="po")
                    nc.tensor.matmul(
                        ps_a[:, :],
                        y_sb[0:DH, nt * P:(nt + 1) * P],
                        wo_sb[0:DH, :],
                        start=True, stop=True)
                    nc.tensor.matmul(
                        ps_b[:, :],
                        y_sb[DH:P, (nt + 1) * P:(nt + 2) * P],
                        wo_sb[DH:P, :],
                        start=True, stop=True)
                    nc.vector.tensor_scalar_mul(
                        ot[:, k * D:(k + 1) * D], ps_a[:, :], rc_sb[:, nt:nt + 1])
                    nc.vector.tensor_scalar_mul(
                        ot[:, (k + 1) * D:(k + 2) * D], ps_b[:, :], rc_sb[:, nt + 1:nt + 2])
                nc.sync.dma_start(
                    out=out_r[:, g * 4:(g + 1) * 4, :],
                    in_=ot.rearrange("p (g d) -> p g d", d=D))


def _build():
    nc = bacc.Bacc("TRN2", target_bir_lowering=False, debug=False)
    vT = nc.dram_tensor("vT", [D, N], FP16, kind="ExternalInput")
    tT = nc.dram_tensor("tT", [D, M], FP16, kind="ExternalInput")
    wq = nc.dram_tensor("wq", [D, DH], FP16, kind="ExternalInput")
    wk = nc.dram_tensor("wk", [D, DH], FP16, kind="ExternalInput")
    wv = nc.dram_tensor("wv", [D, DH], FP16, kind="ExternalInput")
    wo = nc.dram_tensor("wo", [DH, D], FP16, kind="ExternalInput")
    out = nc.dram_tensor("out", [N, D], FP16, kind="ExternalOutput")
    dscr = nc.dram_tensor("dscr", [N], F32)
    with TileContext(nc) as tc:
        _body(tc, nc, vT[:, :], tT[:, :], wq[:, :], wk[:, :], wv[:, :],
              wo[:, :], out[:, :], dscr[:])
    nc.compile()
    return nc


def _pos_encoding():
    # Mirror the reference's jnp ops bit-for-bit (numpy's f32 sin/exp differ
    # by enough ULPs to dominate the error budget at pos/freq ~ 4e3).
    import jax
    import jax.numpy as jnp
    with jax.default_device(jax.devices("cpu")[0]):
        pos = jnp.arange(N, dtype=jnp.float32)
        freq = jnp.exp(
            (jnp.arange(D // 2, dtype=jnp.float32) / D)
            * jnp.log(jnp.float32(10000.0)))
        x = pos[:, None] / freq
        pe = jnp.stack((jnp.sin(x), jnp.cos(x)), axis=-1)
        return np.asarray(pe.reshape(N, D), dtype=np.float32)


def _fp16(a):
    return np.ascontiguousarray(np.asarray(a, dtype=np.float32).astype(np.float16))


def kernel(video_features, text_features, Wq, Wk, Wv, Wout):
    global LAST_RESULT
    if "nc" not in _CACHE:
        _CACHE["nc"] = _build()
        _CACHE["pe"] = _pos_encoding()
    nc = _CACHE["nc"]

    vT = _fp16(np.asarray(video_features, dtype=np.float32).T)
    tT = _fp16(np.asarray(text_features, dtype=np.float32).T)
    Wq = np.asarray(Wq, dtype=np.float32)
    Wk = np.asarray(Wk, dtype=np.float32)
    Wv = np.asarray(Wv, dtype=np.float32)
    Wout = np.asarray(Wout, dtype=np.float32)

    in_maps = []
    for h in range(NCORES):
        in_maps.append({
            "vT": vT,
            "tT": tT,
            "wq": _fp16(Wq[h]),
            "wk": _fp16(Wk[h]),
            "wv": _fp16(Wv[h]),
            "wo": _fp16(Wout[h * DH:(h + 1) * DH, :]),
        })
    res = run_bass_kernel_spmd(nc, in_maps, list(range(NCORES)), trace=TRACE)
    LAST_RESULT = res
    acc = res.results[0]["out"].astype(np.float32)
    for h in range(1, NCORES):
        acc = acc + res.results[h]["out"].astype(np.float32)
    return (acc + _CACHE["pe"]).astype(np.float32)



# revision 4
# speedup vs baseline: 1.3161x; 1.3161x over previous
"""Trainium2 Bass kernel for an 8-head cross-attention block.

Math (per reference):
    Q = video @ Wq[h]           [4096, 64]  per head
    K = text  @ Wk[h]           [1024, 64]
    V = text  @ Wv[h]           [1024, 64]
    att = softmax(Q @ K^T)      [4096, 1024]   (no scaling)
    y_h = att @ V               [4096, 64]
    out = concat_h(y_h) @ Wout + pos_enc(4096, 512)

Sharding: head-parallel over 8 NeuronCores. Core h owns head h and the
matching 64 rows of Wout (row-parallel). The device produces the
UNNORMALIZED projection out_h = (exp(E)@V') @ Wout_h plus the softmax
denominators den_h (per token); since the per-token 1/den scale commutes
with the output projection, the host applies out_h/den_h during the
all-reduce gather (together with the positional encoding).

Device pipeline (single fused loop over 8 chunks of 512 query tokens):
    Qproj(j+1) -> E pairs(j+1) -> exp(j+1)    [PE + ACT]
    PV(j) -> y dup -> out-proj pairs(j) -> cast -> DMA out(j)
All activations stay fp16 ([feature, token] layout, no transposes);
PSUM accumulates fp32. E and out-proj matmuls contract over K=64, so
each pair runs concurrently on the two 64-row PE tiles (T0/T8 row
tiling via base_partition). exp runs as exp(E - 12) on ACT; the shift
cancels in the host-side normalization. The denominator comes free as
a 65th output row of the att@V matmul via a ones-column appended to V.
Elementwise PSUM->SBUF traffic is split across Vector and GpSimd.
"""

import numpy as np

from concourse import bacc
import concourse.mybir as mybir
from concourse.tile import TileContext
from concourse.bass_utils import run_bass_kernel_spmd

N, M, D, H, DH = 4096, 1024, 512, 8, 64
P = 128
NC = 512          # n-chunk width for the attention pipeline
NJ = N // NC      # 8 n-chunks
DC = D // P       # 4 contraction chunks of 128
MT = M // P       # 8 key tiles of 128
F32 = mybir.dt.float32
FP16 = mybir.dt.float16
EXP = mybir.ActivationFunctionType.Exp
EXP_SHIFT = -12.0  # exp(E + shift): keeps exp in fp16 range; cancels in out/den
NCORES = 8

_CACHE: dict = {}
TRACE = False          # test harness can flip this before calling kernel()
LAST_RESULT = None     # BassKernelResults of the last run (for profiling)


def _body(tc, nc, vT, tT, wq, wk, wv, wo, out, den):
    with tc.tile_pool(name="const", bufs=1) as cp, \
         tc.tile_pool(name="pt", bufs=8) as pt_pool, \
         tc.tile_pool(name="ysb", bufs=3) as ysb_pool, \
         tc.tile_pool(name="ot", bufs=2) as ot_pool, \
         tc.tile_pool(name="ps_e", bufs=2, space="PSUM") as e_pool, \
         tc.tile_pool(name="ps_y", bufs=1, space="PSUM") as y_pool, \
         tc.tile_pool(name="ps_o", bufs=2, space="PSUM") as o_pool, \
         tc.tile_pool(name="ps_q", bufs=1, space="PSUM") as q_pool:

        vt_sb = cp.tile([P, DC * N], FP16, tag="vt")
        tt_sb = cp.tile([P, DC * M], FP16, tag="tt")
        wq_sb = cp.tile([P, DC * DH], FP16, tag="wq")
        wk_sb = cp.tile([P, DC * DH], FP16, tag="wk")
        wv_sb = cp.tile([P, DC * DH], FP16, tag="wv")
        wo_sb = cp.tile([P, D], FP16, tag="wo")      # wo duplicated on both halves
        qt_sb = cp.tile([P, N], FP16, tag="qt")      # Q^T duplicated on both halves
        kt_sb = cp.tile([P, M], FP16, tag="kt")      # K^T duplicated on both halves
        v_sb = cp.tile([P, MT * (DH + 1)], FP16, tag="vsb")
        den_sb = cp.tile([1, N], F32, tag="den")
        bias_sb = cp.tile([P, 1], F32, tag="bias")

        # ---- input loads: small operands first so projections start early ----
        tT3 = tT.rearrange("(c p) m -> p c m", p=P)
        nc.sync.dma_start(out=tt_sb.rearrange("p (c m) -> p c m", m=M),
                          in_=tT3)
        nc.sync.dma_start(out=wk_sb.rearrange("p (c e) -> p c e", e=DH),
                          in_=wk.rearrange("(c p) e -> p c e", p=P))
        nc.sync.dma_start(out=wv_sb.rearrange("p (c e) -> p c e", e=DH),
                          in_=wv.rearrange("(c p) e -> p c e", p=P))
        nc.sync.dma_start(out=wq_sb.rearrange("p (c e) -> p c e", e=DH),
                          in_=wq.rearrange("(c p) e -> p c e", p=P))
        nc.sync.dma_start(out=wo_sb[0:DH, :], in_=wo[:, :])
        nc.sync.dma_start(out=wo_sb[DH:P, :], in_=wo[:, :])

        vT3 = vT.rearrange("(c p) n -> p c n", p=P)
        vt3 = vt_sb.rearrange("p (c n) -> p c n", n=N)

        def dma_vt(j):
            sl = slice(j * NC, (j + 1) * NC)
            nc.sync.dma_start(out=vt3[:, :, sl], in_=vT3[:, :, sl])

        dma_vt(0)

        v3 = v_sb.rearrange("p (m e) -> p m e", e=DH + 1)  # [128, 8, 65]
        nc.vector.memset(v3[:, :, DH], 1.0)
        nc.vector.memset(bias_sb[:, :], EXP_SHIFT)

        # ---- K^T [64->128, 1024] and V' [128 keys, 8 x 65] projections ----
        for half in range(M // 512):
            ps = q_pool.tile([P, 512], F32, tag="q")
            sl = slice(half * 512, (half + 1) * 512)
            for c in range(DC):
                nc.tensor.matmul(
                    ps[0:DH, :],
                    wk_sb[:, c * DH:(c + 1) * DH],
                    tt_sb[:, c * M + half * 512: c * M + (half + 1) * 512],
                    start=(c == 0), stop=(c == DC - 1))
            nc.vector.tensor_copy(out=kt_sb[0:DH, sl], in_=ps[0:DH, :])
            nc.vector.tensor_copy(out=kt_sb[DH:P, sl], in_=ps[0:DH, :])
        for mt in range(MT):
            ps = o_pool.tile([P, 512], F32, tag="o")
            for c in range(DC):
                nc.tensor.matmul(
                    ps[:, 0:DH],
                    tt_sb[:, c * M + mt * P: c * M + (mt + 1) * P],
                    wv_sb[:, c * DH:(c + 1) * DH],
                    start=(c == 0), stop=(c == DC - 1))
            nc.vector.tensor_copy(out=v3[:, mt, 0:DH], in_=ps[:, 0:DH])

        dma_vt(1)

        # ---- fused attention + output pipeline over chunks ----
        out_r = out.rearrange("(g p) d -> p g d", p=P)  # [128, 32, 512]
        pts = {}
        for j in range(-1, NJ):
            jq = j + 1
            if jq < NJ:
                if jq + 1 < NJ:
                    dma_vt(jq + 1)
                jqsl = slice(jq * NC, (jq + 1) * NC)
                # Q^T projection for chunk jq
                qp = q_pool.tile([P, 512], F32, tag="q")
                for c in range(DC):
                    nc.tensor.matmul(
                        qp[0:DH, :],
                        wq_sb[:, c * DH:(c + 1) * DH],
                        vt_sb[:, c * N + jq * NC: c * N + (jq + 1) * NC],
                        start=(c == 0), stop=(c == DC - 1))
                nc.vector.tensor_copy(out=qt_sb[0:DH, jqsl], in_=qp[0:DH, :])
                nc.vector.tensor_copy(out=qt_sb[DH:P, jqsl], in_=qp[0:DH, :])
                # E^T pairs (row-tiled, concurrent) + exp
                plist = []
                for pair in range(MT // 2):
                    mt = pair * 2
                    e_ps = e_pool.tile([P, 1024], F32, tag="e")
                    nc.tensor.matmul(
                        e_ps[:, 0:512],
                        kt_sb[0:DH, mt * P:(mt + 1) * P],
                        qt_sb[0:DH, jqsl],
                        start=True, stop=True)
                    nc.tensor.matmul(
                        e_ps[:, 512:1024],
                        kt_sb[DH:P, (mt + 1) * P:(mt + 2) * P],
                        qt_sb[DH:P, jqsl],
                        start=True, stop=True)
                    pt = pt_pool.tile([P, 1024], FP16, tag="p")
                    nc.scalar.activation(pt[:, :], e_ps[:, :], EXP, bias=bias_sb[:, :])
                    plist.append(pt)
                pts[jq] = plist
            if j < 0:
                continue
            jsl = slice(j * NC, (j + 1) * NC)
            # PV: y'^T [65, 512] = V'.T @ P^T, denominator in row 64
            yp = y_pool.tile([DH + 1, 512], F32, tag="y")
            for mt in range(MT):
                nc.tensor.matmul(
                    yp[:, :],
                    v3[:, mt, :],
                    pts[j][mt // 2][:, (mt % 2) * 512:(mt % 2 + 1) * 512],
                    start=(mt == 0), stop=(mt == MT - 1))
            del pts[j]
            nc.vector.tensor_copy(out=den_sb[:, jsl], in_=yp[DH:DH + 1, :])
            ysb = ysb_pool.tile([P, 512], FP16, tag="ysb")
            nc.vector.tensor_copy(out=ysb[0:DH, :], in_=yp[0:DH, :])
            nc.vector.tensor_copy(out=ysb[DH:P, :], in_=yp[0:DH, :])
            # output projection (K=64, row-tiled pairs) + cast + store
            ot = ot_pool.tile([P, 4 * D], FP16, tag="o16")
            for pp in range(2):
                nta = pp * 2
                ps_a = o_pool.tile([P, 512], F32, tag="o")
                ps_b = o_pool.tile([P, 512], F32, tag="o")
                nc.tensor.matmul(
                    ps_a[:, :],
                    ysb[0:DH, nta * P:(nta + 1) * P],
                    wo_sb[0:DH, :],
                    start=True, stop=True)
                nc.tensor.matmul(
                    ps_b[:, :],
                    ysb[DH:P, (nta + 1) * P:(nta + 2) * P],
                    wo_sb[DH:P, :],
                    start=True, stop=True)
                nc.vector.tensor_copy(out=ot[:, nta * D:(nta + 1) * D], in_=ps_a[:, :])
                nc.vector.tensor_copy(out=ot[:, (nta + 1) * D:(nta + 2) * D], in_=ps_b[:, :])
            nc.sync.dma_start(
                out=out_r[:, j * 4:(j + 1) * 4, :],
                in_=ot.rearrange("p (g d) -> p g d", d=D))
        nc.sync.dma_start(out=den[:, :], in_=den_sb[:, :])


def _build():
    nc = bacc.Bacc("TRN2", target_bir_lowering=False, debug=False)
    vT = nc.dram_tensor("vT", [D, N], FP16, kind="ExternalInput")
    tT = nc.dram_tensor("tT", [D, M], FP16, kind="ExternalInput")
    wq = nc.dram_tensor("wq", [D, DH], FP16, kind="ExternalInput")
    wk = nc.dram_tensor("wk", [D, DH], FP16, kind="ExternalInput")
    wv = nc.dram_tensor("wv", [D, DH], FP16, kind="ExternalInput")
    wo = nc.dram_tensor("wo", [DH, D], FP16, kind="ExternalInput")
    out = nc.dram_tensor("out", [N, D], FP16, kind="ExternalOutput")
    den = nc.dram_tensor("den", [1, N], F32, kind="ExternalOutput")
    with TileContext(nc) as tc:
        _body(tc, nc, vT[:, :], tT[:, :], wq[:, :], wk[:, :], wv[:, :],
              wo[:, :], out[:, :], den[:, :])
    nc.compile()
    return nc


def _pos_encoding():
    # Mirror the reference's jnp ops bit-for-bit (numpy's f32 sin/exp differ
    # by enough ULPs to dominate the error budget at pos/freq ~ 4e3).
    import jax
    import jax.numpy as jnp
    with jax.default_device(jax.devices("cpu")[0]):
        pos = jnp.arange(N, dtype=jnp.float32)
        freq = jnp.exp(
            (jnp.arange(D // 2, dtype=jnp.float32) / D)
            * jnp.log(jnp.float32(10000.0)))
        x = pos[:, None] / freq
        pe = jnp.stack((jnp.sin(x), jnp.cos(x)), axis=-1)
        return np.asarray(pe.reshape(N, D), dtype=np.float32)


def _fp16(a):
    return np.ascontiguousarray(np.asarray(a, dtype=np.float32).astype(np.float16))


def kernel(video_features, text_features, Wq, Wk, Wv, Wout):
    global LAST_RESULT
    if "nc" not in _CACHE:
        _CACHE["nc"] = _build()
        _CACHE["pe"] = _pos_encoding()
    nc = _CACHE["nc"]

    vT = _fp16(np.asarray(video_features, dtype=np.float32).T)
    tT = _fp16(np.asarray(text_features, dtype=np.float32).T)
    Wq = np.asarray(Wq, dtype=np.float32)
    Wk = np.asarray(Wk, dtype=np.float32)
    Wv = np.asarray(Wv, dtype=np.float32)
    Wout = np.asarray(Wout, dtype=np.float32)

    in_maps = []
    for h in range(NCORES):
        in_maps.append({
            "vT": vT,
            "tT": tT,
            "wq": _fp16(Wq[h]),
            "wk": _fp16(Wk[h]),
            "wv": _fp16(Wv[h]),
            "wo": _fp16(Wout[h * DH:(h + 1) * DH, :]),
        })
    res = run_bass_kernel_spmd(nc, in_maps, list(range(NCORES)), trace=TRACE)
    LAST_RESULT = res
    acc = None
    for h in range(NCORES):
        o = res.results[h]["out"].astype(np.float32)
        d = res.results[h]["den"].reshape(N, 1)
        part = o / d
        acc = part if acc is None else acc + part
    return (acc + _CACHE["pe"]).astype(np.float32)


# revision 6
# speedup vs baseline: 1.4391x; 1.0934x over previous
"""Trainium2 Bass kernel for an 8-head cross-attention block.

Math (per reference):
    Q = video @ Wq[h]           [4096, 64]  per head
    K = text  @ Wk[h]           [1024, 64]
    V = text  @ Wv[h]           [1024, 64]
    att = softmax(Q @ K^T)      [4096, 1024]   (no scaling)
    y_h = att @ V               [4096, 64]
    out = concat_h(y_h) @ Wout + pos_enc(4096, 512)

Sharding: head-parallel over 8 NeuronCores. Core h owns head h and the
matching 64 rows of Wout (row-parallel). The device produces the
UNNORMALIZED projection out_h = (exp(E)@V') @ Wout_h plus the softmax
denominators den_h (per token); since the per-token 1/den scale commutes
with the output projection, the host applies out_h/den_h during the
all-reduce gather (together with the positional encoding).

Device pipeline (single fused loop over 8 chunks of 512 query tokens):
    Qproj(j+1) -> E pairs(j+1) -> exp(j+1)    [PE + ACT]
    PV(j) -> y dup -> out-proj pairs(j) -> cast -> DMA out(j)
All activations stay fp16 ([feature, token] layout, no transposes);
PSUM accumulates fp32. E and out-proj matmuls contract over K=64, so
each pair runs concurrently on the two 64-row PE tiles (T0/T8 row
tiling via base_partition). exp runs as exp(E - 12) on ACT; the shift
cancels in the host-side normalization. The denominator comes free as
a 65th output row of the att@V matmul via a ones-column appended to V.
Elementwise PSUM->SBUF traffic is split across Vector and GpSimd.
"""

import numpy as np

from concourse import bacc
import concourse.mybir as mybir
from concourse.tile import TileContext
from concourse.bass_utils import run_bass_kernel_spmd

N, M, D, H, DH = 4096, 1024, 512, 8, 64
P = 128
NC = 512          # n-chunk width for the attention pipeline
NJ = N // NC      # 8 n-chunks
DC = D // P       # 4 contraction chunks of 128
MT = M // P       # 8 key tiles of 128
F32 = mybir.dt.float32
FP16 = mybir.dt.float16
EXP = mybir.ActivationFunctionType.Exp
EXP_SHIFT = -12.0  # exp(E + shift): keeps exp in fp16 range; cancels in out/den
NCORES = 8

_CACHE: dict = {}
TRACE = False          # test harness can flip this before calling kernel()
LAST_RESULT = None     # BassKernelResults of the last run (for profiling)


def _body(tc, nc, vT, tT, wq, wk, wv, wo, out, den):
    with tc.tile_pool(name="const", bufs=1) as cp, \
         tc.tile_pool(name="pt", bufs=12) as pt_pool, \
         tc.tile_pool(name="ysb", bufs=3) as ysb_pool, \
         tc.tile_pool(name="ot", bufs=2) as ot_pool, \
         tc.tile_pool(name="ps_e", bufs=2, space="PSUM") as e_pool, \
         tc.tile_pool(name="ps_y", bufs=1, space="PSUM") as y_pool, \
         tc.tile_pool(name="ps_o", bufs=2, space="PSUM") as o_pool, \
         tc.tile_pool(name="ps_q", bufs=1, space="PSUM") as q_pool:

        vt_sb = cp.tile([P, DC * N], FP16, tag="vt")
        tt_sb = cp.tile([P, DC * M], FP16, tag="tt")
        wq_sb = cp.tile([P, DC * DH], FP16, tag="wq")
        wk_sb = cp.tile([P, DC * DH], FP16, tag="wk")
        wv_sb = cp.tile([P, DC * DH], FP16, tag="wv")
        wo_sb = cp.tile([P, D], FP16, tag="wo")      # wo duplicated on both halves
        qt_sb = cp.tile([P, N], FP16, tag="qt")      # Q^T duplicated on both halves
        kt_sb = cp.tile([P, M], FP16, tag="kt")      # K^T duplicated on both halves
        v_sb = cp.tile([P, MT * (DH + 1)], FP16, tag="vsb")
        bias_sb = cp.tile([P, 1], F32, tag="bias")

        # ---- input loads: small operands first so projections start early ----
        tT3 = tT.rearrange("(c p) m -> p c m", p=P)
        nc.sync.dma_start(out=tt_sb.rearrange("p (c m) -> p c m", m=M),
                          in_=tT3)
        nc.sync.dma_start(out=wk_sb.rearrange("p (c e) -> p c e", e=DH),
                          in_=wk.rearrange("(c p) e -> p c e", p=P))
        nc.sync.dma_start(out=wv_sb.rearrange("p (c e) -> p c e", e=DH),
                          in_=wv.rearrange("(c p) e -> p c e", p=P))
        nc.sync.dma_start(out=wq_sb.rearrange("p (c e) -> p c e", e=DH),
                          in_=wq.rearrange("(c p) e -> p c e", p=P))
        nc.sync.dma_start(out=wo_sb[0:DH, :], in_=wo[:, :])
        nc.sync.dma_start(out=wo_sb[DH:P, :], in_=wo[:, :])

        vT3 = vT.rearrange("(c p) n -> p c n", p=P)
        vt3 = vt_sb.rearrange("p (c n) -> p c n", n=N)

        def dma_vt(j):
            sl = slice(j * NC, (j + 1) * NC)
            nc.sync.dma_start(out=vt3[:, :, sl], in_=vT3[:, :, sl])

        dma_vt(0)

        v3 = v_sb.rearrange("p (m e) -> p m e", e=DH + 1)  # [128, 8, 65]
        nc.vector.memset(v3[:, :, DH], 1.0)
        nc.vector.memset(bias_sb[:, :], EXP_SHIFT)

        # ---- K^T [64->128, 1024] and V' [128 keys, 8 x 65] projections ----
        for half in range(M // 512):
            ps = q_pool.tile([P, 512], F32, tag="q")
            sl = slice(half * 512, (half + 1) * 512)
            for c in range(DC):
                nc.tensor.matmul(
                    ps[0:DH, :],
                    wk_sb[:, c * DH:(c + 1) * DH],
                    tt_sb[:, c * M + half * 512: c * M + (half + 1) * 512],
                    start=(c == 0), stop=(c == DC - 1))
            nc.vector.tensor_copy(out=kt_sb[0:DH, sl], in_=ps[0:DH, :])
            nc.vector.tensor_copy(out=kt_sb[DH:P, sl], in_=ps[0:DH, :])
        for mt in range(MT):
            ps = o_pool.tile([P, 512], F32, tag="o")
            for c in range(DC):
                nc.tensor.matmul(
                    ps[:, 0:DH],
                    tt_sb[:, c * M + mt * P: c * M + (mt + 1) * P],
                    wv_sb[:, c * DH:(c + 1) * DH],
                    start=(c == 0), stop=(c == DC - 1))
            nc.vector.tensor_copy(out=v3[:, mt, 0:DH], in_=ps[:, 0:DH])

        dma_vt(1)

        # ---- fused attention + output pipeline over chunks ----
        # Stages are skewed so the PE never waits on ACT/DVE results of the
        # same chunk (continuous PE execution ramps it to the 2.4 GHz
        # p-state): Qproj(t) | E+exp(t-1) | PV(t-3) | out-proj+store(t-4).
        out_r = out.rearrange("(g p) d -> p g d", p=P)  # [128, 32, 512]
        pts = {}
        ysbs = {}
        for t in range(NJ + 4):
            uq = t          # Q^T projection chunk
            ue = t - 1      # E/exp chunk
            up = t - 3      # PV chunk
            uo = t - 4      # out-proj/store chunk
            if uq < NJ:
                if uq + 1 < NJ:
                    dma_vt(uq + 1)
                sl = slice(uq * NC, (uq + 1) * NC)
                qp = q_pool.tile([P, 512], F32, tag="q")
                for c in range(DC):
                    nc.tensor.matmul(
                        qp[0:DH, :],
                        wq_sb[:, c * DH:(c + 1) * DH],
                        vt_sb[:, c * N + uq * NC: c * N + (uq + 1) * NC],
                        start=(c == 0), stop=(c == DC - 1))
                nc.vector.tensor_copy(out=qt_sb[0:DH, sl], in_=qp[0:DH, :])
                nc.gpsimd.dma_start(out=qt_sb[DH:P, sl], in_=qt_sb[0:DH, sl])
            if 0 <= ue < NJ:
                sl = slice(ue * NC, (ue + 1) * NC)
                plist = []
                for pair in range(MT // 2):
                    mt = pair * 2
                    e_ps = e_pool.tile([P, 1024], F32, tag="e")
                    nc.tensor.matmul(
                        e_ps[:, 0:512],
                        kt_sb[0:DH, mt * P:(mt + 1) * P],
                        qt_sb[0:DH, sl],
                        start=True, stop=True)
                    nc.tensor.matmul(
                        e_ps[:, 512:1024],
                        kt_sb[DH:P, (mt + 1) * P:(mt + 2) * P],
                        qt_sb[DH:P, sl],
                        start=True, stop=True)
                    pt = pt_pool.tile([P, 1024], FP16, tag="p")
                    nc.scalar.activation(pt[:, :], e_ps[:, :], EXP, bias=bias_sb[:, :])
                    plist.append(pt)
                pts[ue] = plist
            if 0 <= up < NJ:
                # PV: y'^T [65, 512] = V'.T @ P^T, denominator in row 64.
                yp = y_pool.tile([DH + 1, 512], F32, tag="y")
                for mt in range(MT):
                    nc.tensor.matmul(
                        yp[:, :],
                        v3[:, mt, :],
                        pts[up][mt // 2][:, (mt % 2) * 512:(mt % 2 + 1) * 512],
                        start=(mt == 0), stop=(mt == MT - 1))
                del pts[up]
                # One 65-row cast: y half + fp16 den row; dup the y half to
                # partitions 64-127 (different columns) via SBUF->SBUF DMA.
                ysb = ysb_pool.tile([P, 1024], FP16, tag="ysb")
                nc.vector.tensor_copy(out=ysb[0:DH + 1, 0:512], in_=yp[:, :])
                nc.gpsimd.dma_start(out=ysb[DH:P, 512:1024], in_=ysb[0:DH, 0:512])
                nc.sync.dma_start(out=den[:, up * NC:(up + 1) * NC],
                                  in_=ysb[DH:DH + 1, 0:512])
                ysbs[up] = ysb
            if 0 <= uo:
                ysb = ysbs.pop(uo)
                ot = ot_pool.tile([P, 4 * D], FP16, tag="o16")
                for pp in range(2):
                    nta = pp * 2
                    ps_a = o_pool.tile([P, 512], F32, tag="o")
                    ps_b = o_pool.tile([P, 512], F32, tag="o")
                    nc.tensor.matmul(
                        ps_a[:, :],
                        ysb[0:DH, nta * P:(nta + 1) * P],
                        wo_sb[0:DH, :],
                        start=True, stop=True)
                    nc.tensor.matmul(
                        ps_b[:, :],
                        ysb[DH:P, 512 + (nta + 1) * P: 512 + (nta + 2) * P],
                        wo_sb[DH:P, :],
                        start=True, stop=True)
                    nc.vector.tensor_copy(out=ot[:, nta * D:(nta + 1) * D], in_=ps_a[:, :])
                    nc.vector.tensor_copy(out=ot[:, (nta + 1) * D:(nta + 2) * D], in_=ps_b[:, :])
                nc.sync.dma_start(
                    out=out_r[:, uo * 4:(uo + 1) * 4, :],
                    in_=ot.rearrange("p (g d) -> p g d", d=D))


def _build():
    nc = bacc.Bacc("TRN2", target_bir_lowering=False, debug=False)
    vT = nc.dram_tensor("vT", [D, N], FP16, kind="ExternalInput")
    tT = nc.dram_tensor("tT", [D, M], FP16, kind="ExternalInput")
    wq = nc.dram_tensor("wq", [D, DH], FP16, kind="ExternalInput")
    wk = nc.dram_tensor("wk", [D, DH], FP16, kind="ExternalInput")
    wv = nc.dram_tensor("wv", [D, DH], FP16, kind="ExternalInput")
    wo = nc.dram_tensor("wo", [DH, D], FP16, kind="ExternalInput")
    out = nc.dram_tensor("out", [N, D], FP16, kind="ExternalOutput")
    den = nc.dram_tensor("den", [1, N], FP16, kind="ExternalOutput")
    with TileContext(nc) as tc:
        _body(tc, nc, vT[:, :], tT[:, :], wq[:, :], wk[:, :], wv[:, :],
              wo[:, :], out[:, :], den[:, :])
    nc.compile()
    return nc


def _pos_encoding():
    # Mirror the reference's jnp ops bit-for-bit (numpy's f32 sin/exp differ
    # by enough ULPs to dominate the error budget at pos/freq ~ 4e3).
    import jax
    import jax.numpy as jnp
    with jax.default_device(jax.devices("cpu")[0]):
        pos = jnp.arange(N, dtype=jnp.float32)
        freq = jnp.exp(
            (jnp.arange(D // 2, dtype=jnp.float32) / D)
            * jnp.log(jnp.float32(10000.0)))
        x = pos[:, None] / freq
        pe = jnp.stack((jnp.sin(x), jnp.cos(x)), axis=-1)
        return np.asarray(pe.reshape(N, D), dtype=np.float32)


def _fp16(a):
    return np.ascontiguousarray(np.asarray(a, dtype=np.float32).astype(np.float16))


def kernel(video_features, text_features, Wq, Wk, Wv, Wout):
    global LAST_RESULT
    if "nc" not in _CACHE:
        _CACHE["nc"] = _build()
        _CACHE["pe"] = _pos_encoding()
    nc = _CACHE["nc"]

    vT = _fp16(np.asarray(video_features, dtype=np.float32).T)
    tT = _fp16(np.asarray(text_features, dtype=np.float32).T)
    Wq = np.asarray(Wq, dtype=np.float32)
    Wk = np.asarray(Wk, dtype=np.float32)
    Wv = np.asarray(Wv, dtype=np.float32)
    Wout = np.asarray(Wout, dtype=np.float32)

    in_maps = []
    for h in range(NCORES):
        in_maps.append({
            "vT": vT,
            "tT": tT,
            "wq": _fp16(Wq[h]),
            "wk": _fp16(Wk[h]),
            "wv": _fp16(Wv[h]),
            "wo": _fp16(Wout[h * DH:(h + 1) * DH, :]),
        })
    res = run_bass_kernel_spmd(nc, in_maps, list(range(NCORES)), trace=TRACE)
    LAST_RESULT = res
    acc = None
    for h in range(NCORES):
        o = res.results[h]["out"].astype(np.float32)
        d = res.results[h]["den"].astype(np.float32).reshape(N, 1)
        part = o / d
        acc = part if acc is None else acc + part
    return (acc + _CACHE["pe"]).astype(np.float32)
